# revision 1
# baseline (speedup 1.0000x reference)
"""CTR self-attention kernel for Trainium2 (8 NeuronCores, data-parallel over batch).

Reference computation (per batch b, L=1024, E=O=512, K=4):
    delta = delta_embedding.sum(-1)                       # [L, L]
    valid[i] = i < traj_length[b]
    mask = outer(valid, valid)
    q, k, v = X @ Wq, X @ Wk, X @ Wv                      # [L, O]
    scores = q @ k.T + delta                              # [L, L]
    attn = softmax(scores, axis=-1) * mask                # post-softmax mask
    out = attn @ v                                        # [L, O]

Device mapping (per core: 4 batches):
  - all matmuls on TensorE in float32r (tf32-like, ~11-bit mantissa)
  - delta reduction over K as a one-hot matmul accumulated directly into the
    scores PSUM bank (host pre-transposes delta to [B, L, K, L] so (i,k) is
    the contraction axis and j streams contiguously)
  - softmax: VectorE negated rowmax -> ScalarE Exp(bias=-max) with fused
    row-sum accumulator -> scale folded into the output copy
  - column mask folded into v rows (zero v[j] for j >= t); row mask folded
    into the 1/denominator per-row scale
  - DMA load split across both HWDGE rings (SP + ACT)
"""

import os

import ml_dtypes
import numpy as np

B, L, E, O, KD = 32, 1024, 512, 512, 4
NCORES = 8
BPC = B // NCORES  # batches per core

_compiled = {}


def _build(reps=1, lps=(8, 8, 8, 8)):
    from contextlib import ExitStack

    import concourse.bass as bass
    import concourse.tile as tile
    from concourse import bacc, mybir

    FP32 = mybir.dt.float32
    FP32R = mybir.dt.float32r
    BF16 = mybir.dt.bfloat16
    AX = mybir.AxisListType
    ALU = mybir.AluOpType
    ACTF = mybir.ActivationFunctionType

    nc = bacc.Bacc("TRN2", target_bir_lowering=False, debug=False,
                   num_devices=NCORES)

    NLT = L // 128        # 8 l-tiles (also i-tiles / j-tiles)
    NET = E // 128        # 4 e-tiles
    NOT = O // 128        # 4 o-tiles
    NJC = L // 512        # 2 chunks of 512 along the free dim

    xT_d = nc.dram_tensor("xT", (BPC, E, L), FP32R, kind="ExternalInput")
    dre_d = nc.dram_tensor("dre", (BPC, L * KD, L), BF16, kind="ExternalInput")
    wq_d = nc.dram_tensor("wq", (E, O), FP32R, kind="ExternalInput")
    wk_d = nc.dram_tensor("wk", (E, O), FP32R, kind="ExternalInput")
    wv_d = nc.dram_tensor("wv", (E, O), FP32R, kind="ExternalInput")
    val_d = nc.dram_tensor("val", (BPC, L), FP32, kind="ExternalInput")
    rg_d = nc.dram_tensor("rg", (NLT // 2, 128, 128), BF16, kind="ExternalInput")
    id_d = nc.dram_tensor("ident", (128, 128), FP32R, kind="ExternalInput")
    out_d = nc.dram_tensor("out", (BPC, L, O), FP32, kind="ExternalOutput")

    with tile.TileContext(nc) as tc, ExitStack() as ctx:
        cpool = ctx.enter_context(tc.tile_pool(name="const", bufs=1))
        xpool = ctx.enter_context(tc.tile_pool(name="xt", bufs=2))
        qkpool = ctx.enter_context(tc.tile_pool(name="qk", bufs=2))
        vpool = ctx.enter_context(tc.tile_pool(name="v", bufs=1))
        dpool = ctx.enter_context(tc.tile_pool(name="delta", bufs=12))
        ppool = ctx.enter_context(tc.tile_pool(name="p", bufs=2))
        ptpool = ctx.enter_context(tc.tile_pool(name="pt", bufs=2))
        opool = ctx.enter_context(tc.tile_pool(name="osb", bufs=3))
        smpool = ctx.enter_context(tc.tile_pool(name="small", bufs=8))
        vlpool = ctx.enter_context(tc.tile_pool(name="vl", bufs=2))
        # PSUM: scores/proj accumulators share slots (2 banks x 2), pT
        # transposes (1 bank x 2), attn@v output (1 bank x 2)
        scps = ctx.enter_context(tc.tile_pool(name="scps", bufs=2, space="PSUM"))
        trps = ctx.enter_context(tc.tile_pool(name="trps", bufs=2, space="PSUM"))
        ops = ctx.enter_context(tc.tile_pool(name="ops", bufs=2, space="PSUM"))

        # DMA ring round-robin: SP and ACT HWDGE rings
        rings = [nc.sync, nc.scalar]
        ring_i = [0]

        def dma(out_ap, in_ap):
            eng = rings[ring_i[0] % 2]
            ring_i[0] += 1
            eng.dma_start(out_ap, in_ap)

        # constants
        wq_t = cpool.tile([128, NET, O], FP32R, tag="wq")
        wk_t = cpool.tile([128, NET, O], FP32R, tag="wk")
        wv_t = cpool.tile([128, NET, O], FP32R, tag="wv")
        rg_t = cpool.tile([128, NLT // 2, 128], BF16, tag="rg")
        id_t = cpool.tile([128, 128], FP32R, tag="ident")
        for t, d in ((wq_t, wq_d), (wk_t, wk_d), (wv_t, wv_d)):
            dma(t[:], d[:].rearrange("(et p) o -> p et o", p=128))
        dma(rg_t[:], rg_d[:].rearrange("g p m -> p g m"))
        dma(id_t[:], id_d[:])

        for rep in range(reps):
            for b in range(BPC):
                n_it = lps[b]
                # q columns beyond the live i-tiles are never read
                n_lc_q = min(NJC, (n_it * 128 + 511) // 512)
                # ---- load xT + valid for this batch ----
                xt = xpool.tile([128, NET, L], FP32R, tag="xt")
                for et in range(NET):
                    dma(xt[:, et, :], xT_d[b, et * 128:(et + 1) * 128, :])
                vl = vlpool.tile([128, NLT], FP32, tag="vl")
                dma(vl[:], val_d[b].rearrange("(lt p) -> p lt", p=128))

                # ---- projections (pairs of accumulators -> PSUM-bank
                # alternation keeps the PE pipelined) ----
                qT = qkpool.tile([128, NOT, L], FP32R, tag="qT")
                kT = qkpool.tile([128, NOT, L], FP32R, tag="kT")
                jobs = [(wq_t, qT, ot, lc) for ot in range(NOT)
                        for lc in range(n_lc_q)]
                jobs += [(wk_t, kT, ot, lc) for ot in range(NOT)
                         for lc in range(NJC)]
                for j0 in range(0, len(jobs), 2):
                    pair = jobs[j0:j0 + 2]
                    accs = []
                    for pi, (wt, dst, ot, lc) in enumerate(pair):
                        acc = scps.tile([128, 1024], FP32, tag="sc",
                                        name=f"acc_{b}_{j0}_{pi}")
                        accs.append(acc)
                    for et in range(NET):
                        for pi, (wt, dst, ot, lc) in enumerate(pair):
                            nc.tensor.matmul(
                                accs[pi][:, 0:512],
                                wt[:, et, ot * 128:(ot + 1) * 128],
                                xt[:, et, lc * 512:(lc + 1) * 512],
                                start=(et == 0), stop=(et == NET - 1),
                            )
                    for pi, (wt, dst, ot, lc) in enumerate(pair):
                        nc.vector.tensor_copy(
                            dst[:, ot, lc * 512:(lc + 1) * 512],
                            accs[pi][:, 0:512],
                        )

                v_t = vpool.tile([128, NLT, O], FP32R, tag="v")
                for lt0 in range(0, NLT, 2):
                    accs = []
                    for pi in range(2):
                        acc = scps.tile([128, 1024], FP32, tag="sc",
                                        name=f"vacc_{b}_{lt0}_{pi}")
                        accs.append(acc)
                    for et in range(NET):
                        for pi in range(2):
                            lt = lt0 + pi
                            nc.tensor.matmul(
                                accs[pi][:, 0:512],
                                xt[:, et, lt * 128:(lt + 1) * 128],
                                wv_t[:, et, :],
                                start=(et == 0), stop=(et == NET - 1),
                            )
                    for pi in range(2):
                        lt = lt0 + pi
                        # fold column mask: zero rows j >= t
                        nc.scalar.activation(
                            v_t[:, lt, :], accs[pi][:, 0:512], ACTF.Copy,
                            bias=0.0, scale=vl[:, lt:lt + 1],
                        )

                # ---- attention, one i-tile (128 queries) at a time ----
                for it in range(n_it):
                    # delta tiles first so their DMAs queue ahead
                    dls = []
                    for g in range(NLT // 2):
                        dl = dpool.tile([128, 1024], BF16, tag="dl",
                                        name=f"dl_{b}_{it}_{g}")
                        r0 = (it * 128 + g * 32) * KD
                        dma(dl[:], dre_d[b, r0:r0 + 128, :])
                        dls.append(dl)

                    sc = scps.tile([128, 1024], FP32, tag="sc",
                                   name=f"sc_{b}_{it}")
                    # scores = q @ k.T   (jc inner -> bank alternation)
                    for ot in range(NOT):
                        for jc in range(NJC):
                            nc.tensor.matmul(
                                sc[:, jc * 512:(jc + 1) * 512],
                                qT[:, ot, it * 128:(it + 1) * 128],
                                kT[:, ot, jc * 512:(jc + 1) * 512],
                                start=(ot == 0), stop=False,
                                skip_group_check=True,
                            )
                    # += delta (one-hot reduction over the packed (i,k) axis)
                    for g in range(NLT // 2):
                        for jc in range(NJC):
                            nc.tensor.matmul(
                                sc[:, jc * 512:(jc + 1) * 512],
                                rg_t[:, g, :],
                                dls[g][:, jc * 512:(jc + 1) * 512],
                                start=False, stop=(g == NLT // 2 - 1),
                                skip_group_check=True,
                            )
                    # softmax (denominator over the full row; mask after)
                    nm = smpool.tile([128, 1], FP32, tag="nm",
                                     name=f"nm_{b}_{it}")
                    nc.vector.tensor_reduce(nm[:], sc[:], axis=AX.X,
                                            op=ALU.max, negate=True)
                    p = ppool.tile([128, 1024], FP32R, tag="p",
                                   name=f"p_{b}_{it}")
                    den = smpool.tile([128, 1], FP32, tag="den",
                                      name=f"den_{b}_{it}")
                    nc.scalar.activation(p[:], sc[:], ACTF.Exp, bias=nm[:],
                                         scale=1.0, accum_out=den[:])
                    rs = smpool.tile([128, 1], FP32, tag="rs",
                                     name=f"rs_{b}_{it}")
                    nc.vector.reciprocal(rs[:], den[:])
                    rsm = smpool.tile([128, 1], FP32, tag="rsm",
                                      name=f"rsm_{b}_{it}")
                    nc.vector.tensor_mul(rsm[:], rs[:], vl[:, it:it + 1])

                    # transpose p -> pT, alternating the two PSUM slots
                    pT = ptpool.tile([128, 1024], FP32R, tag="pT",
                                     name=f"pT_{b}_{it}")
                    ptps = []
                    for half in range(2):
                        ptp = trps.tile([128, 512], FP32R, tag="ptp",
                                        name=f"ptp_{b}_{it}_{half}")
                        ptps.append(ptp)
                    for tt in range(4):
                        for half in range(2):
                            jt = half * 4 + tt
                            nc.tensor.transpose(
                                ptps[half][:, tt * 128:(tt + 1) * 128],
                                p[:, jt * 128:(jt + 1) * 128],
                                id_t[:],
                            )
                    for half in range(2):
                        nc.scalar.copy(
                            pT[:, half * 512:(half + 1) * 512], ptps[half][:]
                        )

                    # out = (p/den * mask) @ v
                    op = ops.tile([128, 512], FP32, tag="op",
                                  name=f"op_{b}_{it}")
                    for jt in range(NLT):
                        nc.tensor.matmul(
                            op[:],
                            pT[:, jt * 128:(jt + 1) * 128],
                            v_t[:, jt, :],
                            start=(jt == 0), stop=(jt == NLT - 1),
                        )
                    ob = opool.tile([128, 512], FP32, tag="ob",
                                    name=f"ob_{b}_{it}")
                    nc.vector.tensor_scalar(ob[:], op[:], rsm[:], None,
                                            op0=ALU.mult)
                    nc.sync.dma_start(
                        out_d[b, it * 128:(it + 1) * 128, :], ob[:]
                    )

    nc.compile()
    return nc


def _get_compiled(lps=(8, 8, 8, 8)):
    lps = tuple(lps)
    if lps not in _compiled:
        _compiled[lps] = _build(
            reps=int(os.environ.get("CTR_KERNEL_REPS", "1")), lps=lps)
    return _compiled[lps]


def _schedule(traj_length):
    """Assign batches to (core, position) so that position-wise max live
    i-tile counts are minimal; returns (perm, lps) with perm[c][p] = batch."""
    traj = np.asarray(traj_length)
    live = np.minimum((traj.astype(np.int64) + 127) // 128, L // 128)
    order = np.argsort(-live, kind="stable")
    perm = np.empty((NCORES, BPC), dtype=np.int64)
    lps = []
    for p in range(BPC):
        ranks = order[p * NCORES:(p + 1) * NCORES]
        perm[:, p] = ranks
        lps.append(int(live[ranks].max()))
    return perm, tuple(lps)


def _host_prep(joint_embedding, delta_embedding, Wq, Wk, Wv, traj_length):
    joint_embedding = np.asarray(joint_embedding, dtype=np.float32)
    delta_embedding = np.asarray(delta_embedding, dtype=np.float32)
    valid = (np.arange(L)[None, :] < np.asarray(traj_length)[:, None]
             ).astype(np.float32)
    perm, lps = _schedule(traj_length)

    rg = np.zeros((4, 128, 128), dtype=ml_dtypes.bfloat16)
    for ii in range(32):
        for g in range(4):
            rg[g, ii * KD:(ii + 1) * KD, g * 32 + ii] = 1.0
    ident = np.eye(128, dtype=np.float32)

    in_maps = []
    for c in range(NCORES):
        bs = perm[c]
        xT = np.ascontiguousarray(
            joint_embedding[bs].transpose(0, 2, 1))
        dre = np.ascontiguousarray(
            delta_embedding[bs].transpose(0, 1, 3, 2)
        ).reshape(BPC, L * KD, L).astype(ml_dtypes.bfloat16)
        in_maps.append({
            "xT": xT,
            "dre": dre,
            "wq": np.asarray(Wq, dtype=np.float32),
            "wk": np.asarray(Wk, dtype=np.float32),
            "wv": np.asarray(Wv, dtype=np.float32),
            "val": valid[bs],
            "rg": rg, "ident": ident,
        })
    return in_maps


def kernel(joint_embedding, delta_embedding, Wq, Wk, Wv, traj_length):
    from concourse.bass_utils import run_bass_kernel_spmd

    perm, lps = _schedule(traj_length)
    nc = _get_compiled(lps)
    in_maps = _host_prep(joint_embedding, delta_embedding, Wq, Wk, Wv,
                         traj_length)
    res = run_bass_kernel_spmd(nc, in_maps, core_ids=list(range(NCORES)))
    out = np.empty((B, L, O), dtype=np.float32)
    for c in range(NCORES):
        for p in range(BPC):
            out[perm[c][p]] = res.results[c]["out"][p]
    return out



# revision 2
# speedup vs baseline: 3250.5627x; 3250.5627x over previous
"""CTR self-attention kernel v2 for Trainium2 (8 NeuronCores, data-parallel).

Reference computation (per batch b, L=1024, E=O=512, K=4):
    delta = delta_embedding.sum(-1)                       # [L, L]
    valid[i] = i < traj_length[b]
    mask = outer(valid, valid)
    q, k, v = X @ Wq, X @ Wk, X @ Wv                      # [L, O]
    scores = q @ k.T + delta                              # [L, L]
    attn = softmax(scores, axis=-1) * mask                # post-softmax mask
    out = attn @ v                                        # [L, O]

v2 changes vs v1:
  - delta summed over K on host -> [L, L] fp16; added to the scores PSUM via
    an identity matmul (1024 PE rows/i-tile instead of 4096) and half the
    DMA bytes of the bf16 [L, K, L] layout
  - all matmul operands fp16 (same 11-bit mantissa as fp32r, full PE rate)
  - column mask exploited structurally: v rows j >= t are zero, so attn@v
    and the p-transposes skip j-tiles >= ceil(t/128) entirely; q projection
    and the whole attention i-loop skip i-tiles >= ceil(t/128)
  - output stored fp16 (host converts to fp32); halves the store traffic
  - valid flags packed [128, BPC*8] contiguous on host (one small DMA)
"""

import os

import numpy as np

B, L, E, O, KD = 32, 1024, 1024 // 2, 512, 4
NCORES = 8
BPC = B // NCORES  # batches per core

NLT = L // 128   # 8 l-tiles
NET = E // 128   # 4 e-tiles
NOT = O // 128   # 4 o-tiles
NJC = L // 512   # 2 chunks of 512 along the free dim

_compiled = {}


def _chunks(total, maxc=512):
    out = []
    pos = 0
    while pos < total:
        c = min(maxc, total - pos)
        out.append((pos, c))
        pos += c
    return out


def _build(reps=1, lps=(8, 8, 8, 8)):
    from contextlib import ExitStack

    import concourse.bass as bass
    import concourse.tile as tile
    from concourse import bacc, mybir

    FP32 = mybir.dt.float32
    FP16 = mybir.dt.float16
    AX = mybir.AxisListType
    ALU = mybir.AluOpType
    ACTF = mybir.ActivationFunctionType

    nc = bacc.Bacc("TRN2", target_bir_lowering=False, debug=False,
                   num_devices=NCORES)

    xT_d = nc.dram_tensor("xT", (BPC, E, L), FP16, kind="ExternalInput")
    ds_d = nc.dram_tensor("ds", (BPC, L, L), FP16, kind="ExternalInput")
    wq_d = nc.dram_tensor("wq", (E, O), FP16, kind="ExternalInput")
    wk_d = nc.dram_tensor("wk", (E, O), FP16, kind="ExternalInput")
    wv_d = nc.dram_tensor("wv", (E, O), FP16, kind="ExternalInput")
    val_d = nc.dram_tensor("val", (128, BPC, NLT), FP32, kind="ExternalInput")
    id_d = nc.dram_tensor("ident", (128, 128), FP16, kind="ExternalInput")
    out_d = nc.dram_tensor("out", (BPC, L, O), FP16, kind="ExternalOutput")

    with tile.TileContext(nc) as tc, ExitStack() as ctx:
        cpool = ctx.enter_context(tc.tile_pool(name="const", bufs=1))
        xpool = ctx.enter_context(tc.tile_pool(name="xt", bufs=2))
        qkpool = ctx.enter_context(tc.tile_pool(name="qk", bufs=2))
        vpool = ctx.enter_context(tc.tile_pool(name="v", bufs=2))
        dpool = ctx.enter_context(tc.tile_pool(name="delta", bufs=6))
        ppool = ctx.enter_context(tc.tile_pool(name="p", bufs=2))
        ptpool = ctx.enter_context(tc.tile_pool(name="pt", bufs=2))
        opool = ctx.enter_context(tc.tile_pool(name="osb", bufs=3))
        smpool = ctx.enter_context(tc.tile_pool(name="small", bufs=8))
        # PSUM: scores accumulators 2x[128,1024]f32 (4 banks), transposes
        # 2x[128,1024]f16 (2 banks), attn@v out 2x[128,512]f32 (2 banks)
        scps = ctx.enter_context(tc.tile_pool(name="scps", bufs=2, space="PSUM"))
        trps = ctx.enter_context(tc.tile_pool(name="trps", bufs=2, space="PSUM"))
        ops = ctx.enter_context(tc.tile_pool(name="ops", bufs=2, space="PSUM"))

        rings = [nc.sync, nc.scalar]
        ring_i = [0]

        def dma(out_ap, in_ap):
            eng = rings[ring_i[0] % 2]
            ring_i[0] += 1
            eng.dma_start(out_ap, in_ap)

        # constants
        wq_t = cpool.tile([128, NET, O], FP16, tag="wq")
        wk_t = cpool.tile([128, NET, O], FP16, tag="wk")
        wv_t = cpool.tile([128, NET, O], FP16, tag="wv")
        id_t = cpool.tile([128, 128], FP16, tag="ident")
        vl = cpool.tile([128, BPC, NLT], FP32, tag="vl")
        for t, d in ((wq_t, wq_d), (wk_t, wk_d), (wv_t, wv_d)):
            dma(t[:], d[:].rearrange("(et p) o -> p et o", p=128))
        dma(id_t[:], id_d[:])
        dma(vl[:], val_d[:])

        for rep in range(reps):
            for b in range(BPC):
                live = lps[b]
                # ---- load xT for this batch ----
                xt = xpool.tile([128, NET, L], FP16, tag="xt")
                for et in range(NET):
                    dma(xt[:, et, :], xT_d[b, et * 128:(et + 1) * 128, :])

                # ---- projections (paired accumulators alternate PSUM
                # banks so the PE stays pipelined) ----
                qT = qkpool.tile([128, NOT, L], FP16, tag="qT")
                kT = qkpool.tile([128, NOT, L], FP16, tag="kT")
                v_t = vpool.tile([128, NLT, O], FP16, tag="v")
                qch = _chunks(live * 128)
                jobs = [("q", ot, c0, cn) for ot in range(NOT)
                        for (c0, cn) in qch]
                jobs += [("k", ot, jc * 512, 512) for ot in range(NOT)
                         for jc in range(NJC)]
                jobs += [("v", lt, 0, 512) for lt in range(live)]
                for j0 in range(0, len(jobs), 2):
                    pair = jobs[j0:j0 + 2]
                    accs = []
                    for pi in range(len(pair)):
                        acc = scps.tile([128, 1024], FP32, tag="sc",
                                        name=f"acc_{b}_{j0}_{pi}")
                        accs.append(acc)
                    for et in range(NET):
                        for pi, (kind, a1, c0, cn) in enumerate(pair):
                            if kind == "q":
                                nc.tensor.matmul(
                                    accs[pi][:, 0:cn],
                                    wq_t[:, et, a1 * 128:(a1 + 1) * 128],
                                    xt[:, et, c0:c0 + cn],
                                    start=(et == 0), stop=(et == NET - 1),
                                )
                            elif kind == "k":
                                nc.tensor.matmul(
                                    accs[pi][:, 0:cn],
                                    wk_t[:, et, a1 * 128:(a1 + 1) * 128],
                                    xt[:, et, c0:c0 + cn],
                                    start=(et == 0), stop=(et == NET - 1),
                                )
                            else:
                                nc.tensor.matmul(
                                    accs[pi][:, 0:512],
                                    xt[:, et, a1 * 128:(a1 + 1) * 128],
                                    wv_t[:, et, :],
                                    start=(et == 0), stop=(et == NET - 1),
                                )
                    for pi, (kind, a1, c0, cn) in enumerate(pair):
                        if kind == "q":
                            nc.vector.tensor_copy(
                                qT[:, a1, c0:c0 + cn], accs[pi][:, 0:cn])
                        elif kind == "k":
                            nc.vector.tensor_copy(
                                kT[:, a1, c0:c0 + cn], accs[pi][:, 0:cn])
                        else:
                            # fold column mask: zero rows j >= t in the
                            # (only partial) last live tile
                            nc.scalar.activation(
                                v_t[:, a1, :], accs[pi][:, 0:512], ACTF.Copy,
                                bias=0.0, scale=vl[:, b, a1:a1 + 1],
                            )

                # ---- attention, one i-tile (128 queries) at a time ----
                for it in range(live):
                    dl = dpool.tile([128, 1024], FP16, tag="dl",
                                    name=f"dl_{b}_{it}")
                    dma(dl[:], ds_d[b, it * 128:(it + 1) * 128, :])

                    sc = scps.tile([128, 1024], FP32, tag="sc",
                                   name=f"sc_{b}_{it}")
                    # scores = q @ k.T
                    for ot in range(NOT):
                        for jc in range(NJC):
                            nc.tensor.matmul(
                                sc[:, jc * 512:(jc + 1) * 512],
                                qT[:, ot, it * 128:(it + 1) * 128],
                                kT[:, ot, jc * 512:(jc + 1) * 512],
                                start=(ot == 0), stop=False,
                                skip_group_check=True,
                            )
                    # += delta via identity matmul
                    for jc in range(NJC):
                        nc.tensor.matmul(
                            sc[:, jc * 512:(jc + 1) * 512],
                            id_t[:],
                            dl[:, jc * 512:(jc + 1) * 512],
                            start=False, stop=True,
                            skip_group_check=True,
                        )
                    # softmax over the full row; mask folded in afterwards
                    nm = smpool.tile([128, 1], FP32, tag="nm",
                                     name=f"nm_{b}_{it}")
                    nc.vector.tensor_reduce(nm[:], sc[:], axis=AX.X,
                                            op=ALU.max, negate=True)
                    p = ppool.tile([128, 1024], FP16, tag="p",
                                   name=f"p_{b}_{it}")
                    den = smpool.tile([128, 1], FP32, tag="den",
                                      name=f"den_{b}_{it}")
                    nc.scalar.activation(p[:], sc[:], ACTF.Exp, bias=nm[:],
                                         scale=1.0, accum_out=den[:])
                    rs = smpool.tile([128, 1], FP32, tag="rs",
                                     name=f"rs_{b}_{it}")
                    nc.vector.reciprocal(rs[:], den[:])
                    rsm = smpool.tile([128, 1], FP32, tag="rsm",
                                      name=f"rsm_{b}_{it}")
                    nc.vector.tensor_mul(rsm[:], rs[:], vl[:, b, it:it + 1])

                    # transpose live j-tiles of p into one fp16 PSUM bank
                    ptp = trps.tile([128, 1024], FP16, tag="ptp",
                                    name=f"ptp_{b}_{it}")
                    for jt in range(live):
                        nc.tensor.transpose(
                            ptp[:, jt * 128:(jt + 1) * 128],
                            p[:, jt * 128:(jt + 1) * 128],
                            id_t[:],
                        )
                    pT = ptpool.tile([128, 1024], FP16, tag="pT",
                                     name=f"pT_{b}_{it}")
                    nc.scalar.copy(pT[:, 0:live * 128], ptp[:, 0:live * 128])

                    # out = (p/den * rowmask) @ v -- only live j-tiles
                    op = ops.tile([128, 512], FP32, tag="op",
                                  name=f"op_{b}_{it}")
                    for jt in range(live):
                        nc.tensor.matmul(
                            op[:],
                            pT[:, jt * 128:(jt + 1) * 128],
                            v_t[:, jt, :],
                            start=(jt == 0), stop=(jt == live - 1),
                        )
                    ob = opool.tile([128, 512], FP16, tag="ob",
                                    name=f"ob_{b}_{it}")
                    nc.vector.tensor_scalar(ob[:], op[:], rsm[:], None,
                                            op0=ALU.mult)
                    nc.sync.dma_start(
                        out_d[b, it * 128:(it + 1) * 128, :], ob[:]
                    )

    nc.compile()
    return nc


def _get_compiled(lps=(8, 8, 8, 8)):
    lps = tuple(lps)
    if lps not in _compiled:
        _compiled[lps] = _build(
            reps=int(os.environ.get("CTR_KERNEL_REPS", "1")), lps=lps)
    return _compiled[lps]


def _schedule(traj_length):
    """Assign batches to (core, position) so that position-wise max live
    i-tile counts are minimal; returns (perm, lps) with perm[c][p] = batch."""
    traj = np.asarray(traj_length)
    live = np.minimum((traj.astype(np.int64) + 127) // 128, L // 128)
    order = np.argsort(-live, kind="stable")
    perm = np.empty((NCORES, BPC), dtype=np.int64)
    lps = []
    for p in range(BPC):
        ranks = order[p * NCORES:(p + 1) * NCORES]
        perm[:, p] = ranks
        lps.append(int(live[ranks].max()))
    return perm, tuple(lps)


def _host_prep(joint_embedding, delta_embedding, Wq, Wk, Wv, traj_length):
    joint_embedding = np.asarray(joint_embedding, dtype=np.float32)
    delta_embedding = np.asarray(delta_embedding, dtype=np.float32)
    valid = (np.arange(L)[None, :] < np.asarray(traj_length)[:, None]
             ).astype(np.float32)
    perm, lps = _schedule(traj_length)

    dsum = delta_embedding.sum(axis=-1, dtype=np.float32).astype(np.float16)
    x16 = joint_embedding.astype(np.float16)
    ident = np.eye(128, dtype=np.float16)
    wq16 = np.asarray(Wq, dtype=np.float16)
    wk16 = np.asarray(Wk, dtype=np.float16)
    wv16 = np.asarray(Wv, dtype=np.float16)

    in_maps = []
    for c in range(NCORES):
        bs = perm[c]
        xT = np.ascontiguousarray(x16[bs].transpose(0, 2, 1))
        ds = np.ascontiguousarray(dsum[bs])
        valp = np.ascontiguousarray(
            valid[bs].reshape(BPC, NLT, 128).transpose(2, 0, 1))
        in_maps.append({
            "xT": xT,
            "ds": ds,
            "wq": wq16, "wk": wk16, "wv": wv16,
            "val": valp,
            "ident": ident,
        })
    return in_maps


def kernel(joint_embedding, delta_embedding, Wq, Wk, Wv, traj_length):
    from concourse.bass_utils import run_bass_kernel_spmd

    perm, lps = _schedule(traj_length)
    nc = _get_compiled(lps)
    in_maps = _host_prep(joint_embedding, delta_embedding, Wq, Wk, Wv,
                         traj_length)
    res = run_bass_kernel_spmd(nc, in_maps, core_ids=list(range(NCORES)))
    out = np.empty((B, L, O), dtype=np.float32)
    for c in range(NCORES):
        for p in range(BPC):
            out[perm[c][p]] = res.results[c]["out"][p].astype(np.float32)
    return out


# revision 3
# speedup vs baseline: 3534.5353x; 1.0874x over previous
"""CTR self-attention kernel for Trainium2 (8 NeuronCores, data-parallel
over batch; 4 batches per core, scheduled so per-position live-tile maxima
are minimal).

Reference computation (per batch b, L=1024, E=O=512, K=4):
    delta = delta_embedding.sum(-1)                       # [L, L]
    valid[i] = i < traj_length[b]
    mask = outer(valid, valid)
    q, k, v = X @ Wq, X @ Wk, X @ Wv                      # [L, O]
    scores = q @ k.T + delta                              # [L, L]
    attn = softmax(scores, axis=-1) * mask                # post-softmax mask
    out = attn @ v                                        # [L, O]

Design:
  - delta summed over K on the host -> [L, L] fp16 (4x less HBM traffic than
    the [L, K, L] bf16 layout); added into the scores PSUM group via an
    identity matmul (1024 PE rows/i-tile instead of 4096)
  - every matmul operand is fp16: same 11-bit mantissa as fp32r, so no
    accuracy loss vs the fp32r baseline, but half the SBUF/HBM bytes, FWL
    weight loads (fp32r disqualifies FWL), and no fp32 streaming penalty
  - mask structure exploited: with t = traj_length, live = ceil(t/128),
    q-projection / attention i-tiles / p-transposes / attn@v j-tiles all
    process only live tiles (k and the scores row stay full width: the
    post-softmax mask means the denominator includes all 1024 columns);
    column mask folded into v (zeroed rows), row mask into the 1/den scale
  - softmax: DVE negated rowmax -> ScalarE Exp(bias=-max) with fused row-sum
    accumulator -> reciprocal; p written as fp16, transposed 128x128 on the
    PE into one fp16 PSUM bank, one ScalarE copy out
  - PSUM: 3x[128,1024]f32 score accumulators (6 banks; also reused by the
    projection jobs) + a shared 2-slot pool for transpose/attn@v tiles
  - output stored fp16, widened to fp32 on the host
  - DMA: both HWDGE rings round-robin; loads ordered by first use; delta
    prefetched 8 i-tiles deep
"""

import os

import numpy as np

B, L, E, O, KD = 32, 1024, 1024 // 2, 512, 4
NCORES = 8
BPC = B // NCORES  # batches per core

NLT = L // 128   # 8 l-tiles
NET = E // 128   # 4 e-tiles
NOT = O // 128   # 4 o-tiles
NJC = L // 512   # 2 chunks of 512 along the free dim

_compiled = {}


def _chunks(total, maxc=512):
    out = []
    pos = 0
    while pos < total:
        c = min(maxc, total - pos)
        out.append((pos, c))
        pos += c
    return out


def _build(reps=1, lps=(8, 8, 8, 8)):
    from contextlib import ExitStack

    import concourse.bass as bass
    import concourse.tile as tile
    from concourse import bacc, mybir

    FP32 = mybir.dt.float32
    FP16 = mybir.dt.float16
    AX = mybir.AxisListType
    ALU = mybir.AluOpType
    ACTF = mybir.ActivationFunctionType

    nc = bacc.Bacc("TRN2", target_bir_lowering=False, debug=False,
                   num_devices=NCORES)

    xT_d = nc.dram_tensor("xT", (BPC, E, L), FP16, kind="ExternalInput")
    ds_d = nc.dram_tensor("ds", (BPC, L, L), FP16, kind="ExternalInput")
    wq_d = nc.dram_tensor("wq", (E, O), FP16, kind="ExternalInput")
    wk_d = nc.dram_tensor("wk", (E, O), FP16, kind="ExternalInput")
    wv_d = nc.dram_tensor("wv", (E, O), FP16, kind="ExternalInput")
    val_d = nc.dram_tensor("val", (128, BPC, NLT), FP32, kind="ExternalInput")
    id_d = nc.dram_tensor("ident", (128, 128), FP16, kind="ExternalInput")
    out_d = nc.dram_tensor("out", (BPC, L, O), FP16, kind="ExternalOutput")

    with tile.TileContext(nc) as tc, ExitStack() as ctx:
        cpool = ctx.enter_context(tc.tile_pool(name="const", bufs=1))
        xpool = ctx.enter_context(tc.tile_pool(name="xt", bufs=2))
        qkpool = ctx.enter_context(tc.tile_pool(name="qk", bufs=2))
        vpool = ctx.enter_context(tc.tile_pool(name="v", bufs=2))
        dpool = ctx.enter_context(tc.tile_pool(name="delta", bufs=8))
        ppool = ctx.enter_context(tc.tile_pool(name="p", bufs=3))
        ptpool = ctx.enter_context(tc.tile_pool(name="pt", bufs=3))
        opool = ctx.enter_context(tc.tile_pool(name="osb", bufs=3))
        smpool = ctx.enter_context(tc.tile_pool(name="small", bufs=8))
        # PSUM: scores accumulators 2x[128,1024]f32 (4 banks), transposes
        # 2x[128,1024]f16 (2 banks), attn@v out 2x[128,512]f32 (2 banks)
        scps = ctx.enter_context(tc.tile_pool(name="scps", bufs=3, space="PSUM"))
        trps = ctx.enter_context(tc.tile_pool(name="trps", bufs=2, space="PSUM"))
        ops = trps

        rings = [nc.sync, nc.scalar]
        ring_i = [0]

        def dma(out_ap, in_ap):
            eng = rings[ring_i[0] % 2]
            ring_i[0] += 1
            eng.dma_start(out_ap, in_ap)

        # first batch's xT first: the first projection matmuls need
        # wq + xt[et0]; issuing xt ahead of the bulk weight loads trims the
        # startup bubble
        xt0 = xpool.tile([128, NET, L], FP16, tag="xt")
        wq_t = cpool.tile([128, NET, O], FP16, tag="wq")
        wk_t = cpool.tile([128, NET, O], FP16, tag="wk")
        wv_t = cpool.tile([128, NET, O], FP16, tag="wv")
        id_t = cpool.tile([128, 128], FP16, tag="ident")
        vl = cpool.tile([128, BPC, NLT], FP32, tag="vl")
        # load order tracks first use: q projections (wq + xt0) start the
        # kernel, k projections next, v/ident/val last
        dma(xt0[:, 0, :], xT_d[0, 0:128, :])
        dma(wq_t[:], wq_d[:].rearrange("(et p) o -> p et o", p=128))
        dma(wk_t[:], wk_d[:].rearrange("(et p) o -> p et o", p=128))
        for et in range(1, NET):
            dma(xt0[:, et, :], xT_d[0, et * 128:(et + 1) * 128, :])
        dma(wv_t[:], wv_d[:].rearrange("(et p) o -> p et o", p=128))
        dma(id_t[:], id_d[:])
        dma(vl[:], val_d[:])

        for rep in range(reps):
            for b in range(BPC):
                live = lps[b]
                # ---- load xT for this batch ----
                if rep == 0 and b == 0:
                    xt = xt0
                else:
                    xt = xpool.tile([128, NET, L], FP16, tag="xt")
                    for et in range(NET):
                        dma(xt[:, et, :], xT_d[b, et * 128:(et + 1) * 128, :])

                # ---- projections (paired accumulators alternate PSUM
                # banks so the PE stays pipelined) ----
                qT = qkpool.tile([128, NOT, L], FP16, tag="qT")
                kT = qkpool.tile([128, NOT, L], FP16, tag="kT")
                v_t = vpool.tile([128, NLT, O], FP16, tag="v")
                qch = _chunks(live * 128)
                jobs = [("q", ot, c0, cn) for ot in range(NOT)
                        for (c0, cn) in qch]
                jobs += [("k", ot, jc * 512, 512) for ot in range(NOT)
                         for jc in range(NJC)]
                jobs += [("v", lt, 0, 512) for lt in range(live)]
                for j0 in range(0, len(jobs), 2):
                    pair = jobs[j0:j0 + 2]
                    accs = []
                    for pi in range(len(pair)):
                        acc = scps.tile([128, 1024], FP32, tag="sc",
                                        name=f"acc_{b}_{j0}_{pi}")
                        accs.append(acc)
                    for et in range(NET):
                        for pi, (kind, a1, c0, cn) in enumerate(pair):
                            if kind == "q":
                                nc.tensor.matmul(
                                    accs[pi][:, 0:cn],
                                    wq_t[:, et, a1 * 128:(a1 + 1) * 128],
                                    xt[:, et, c0:c0 + cn],
                                    start=(et == 0), stop=(et == NET - 1),
                                )
                            elif kind == "k":
                                nc.tensor.matmul(
                                    accs[pi][:, 0:cn],
                                    wk_t[:, et, a1 * 128:(a1 + 1) * 128],
                                    xt[:, et, c0:c0 + cn],
                                    start=(et == 0), stop=(et == NET - 1),
                                )
                            else:
                                nc.tensor.matmul(
                                    accs[pi][:, 0:512],
                                    xt[:, et, a1 * 128:(a1 + 1) * 128],
                                    wv_t[:, et, :],
                                    start=(et == 0), stop=(et == NET - 1),
                                )
                    for pi, (kind, a1, c0, cn) in enumerate(pair):
                        if kind == "q":
                            nc.vector.tensor_copy(
                                qT[:, a1, c0:c0 + cn], accs[pi][:, 0:cn])
                        elif kind == "k":
                            nc.vector.tensor_copy(
                                kT[:, a1, c0:c0 + cn], accs[pi][:, 0:cn])
                        else:
                            # fold column mask: zero rows j >= t in the
                            # (only partial) last live tile
                            nc.scalar.activation(
                                v_t[:, a1, :], accs[pi][:, 0:512], ACTF.Copy,
                                bias=0.0, scale=vl[:, b, a1:a1 + 1],
                            )

                # ---- attention, one i-tile (128 queries) at a time ----
                for it in range(live):
                    dl = dpool.tile([128, 1024], FP16, tag="dl",
                                    name=f"dl_{b}_{it}")
                    dma(dl[:], ds_d[b, it * 128:(it + 1) * 128, :])

                    sc = scps.tile([128, 1024], FP32, tag="sc",
                                   name=f"sc_{b}_{it}")
                    # scores = q @ k.T
                    for ot in range(NOT):
                        for jc in range(NJC):
                            nc.tensor.matmul(
                                sc[:, jc * 512:(jc + 1) * 512],
                                qT[:, ot, it * 128:(it + 1) * 128],
                                kT[:, ot, jc * 512:(jc + 1) * 512],
                                start=(ot == 0), stop=False,
                                skip_group_check=True,
                            )
                    # += delta via identity matmul
                    for jc in range(NJC):
                        nc.tensor.matmul(
                            sc[:, jc * 512:(jc + 1) * 512],
                            id_t[:],
                            dl[:, jc * 512:(jc + 1) * 512],
                            start=False, stop=True,
                            skip_group_check=True,
                        )
                    # softmax over the full row; mask folded in afterwards
                    nm = smpool.tile([128, 1], FP32, tag="nm",
                                     name=f"nm_{b}_{it}")
                    nc.vector.tensor_reduce(nm[:], sc[:], axis=AX.X,
                                            op=ALU.max, negate=True)
                    p = ppool.tile([128, 1024], FP16, tag="p",
                                   name=f"p_{b}_{it}")
                    den = smpool.tile([128, 1], FP32, tag="den",
                                      name=f"den_{b}_{it}")
                    nc.scalar.activation(p[:], sc[:], ACTF.Exp, bias=nm[:],
                                         scale=1.0, accum_out=den[:])
                    rs = smpool.tile([128, 1], FP32, tag="rs",
                                     name=f"rs_{b}_{it}")
                    nc.vector.reciprocal(rs[:], den[:])
                    rsm = smpool.tile([128, 1], FP32, tag="rsm",
                                      name=f"rsm_{b}_{it}")
                    nc.vector.tensor_mul(rsm[:], rs[:], vl[:, b, it:it + 1])

                    # transpose live j-tiles of p into one fp16 PSUM bank
                    ptp = trps.tile([128, 1024], FP16, tag="ptp",
                                    name=f"ptp_{b}_{it}")
                    for jt in range(live):
                        nc.tensor.transpose(
                            ptp[:, jt * 128:(jt + 1) * 128],
                            p[:, jt * 128:(jt + 1) * 128],
                            id_t[:],
                        )
                    pT = ptpool.tile([128, 1024], FP16, tag="pT",
                                     name=f"pT_{b}_{it}")
                    nc.scalar.copy(pT[:, 0:live * 128], ptp[:, 0:live * 128])

                    # out = (p/den * rowmask) @ v -- only live j-tiles
                    op = ops.tile([128, 512], FP32, tag="ptp",
                                  name=f"op_{b}_{it}")
                    for jt in range(live):
                        nc.tensor.matmul(
                            op[:],
                            pT[:, jt * 128:(jt + 1) * 128],
                            v_t[:, jt, :],
                            start=(jt == 0), stop=(jt == live - 1),
                        )
                    ob = opool.tile([128, 512], FP16, tag="ob",
                                    name=f"ob_{b}_{it}")
                    nc.vector.tensor_scalar(ob[:], op[:], rsm[:], None,
                                            op0=ALU.mult)
                    nc.sync.dma_start(
                        out_d[b, it * 128:(it + 1) * 128, :], ob[:]
                    )

    nc.compile()
    return nc


def _get_compiled(lps=(8, 8, 8, 8)):
    lps = tuple(lps)
    if lps not in _compiled:
        _compiled[lps] = _build(
            reps=int(os.environ.get("CTR_KERNEL_REPS", "1")), lps=lps)
    return _compiled[lps]


def _schedule(traj_length):
    """Assign batches to (core, position) so that position-wise max live
    i-tile counts are minimal; returns (perm, lps) with perm[c][p] = batch."""
    traj = np.asarray(traj_length)
    live = np.minimum((traj.astype(np.int64) + 127) // 128, L // 128)
    order = np.argsort(-live, kind="stable")
    perm = np.empty((NCORES, BPC), dtype=np.int64)
    lps = []
    for p in range(BPC):
        ranks = order[p * NCORES:(p + 1) * NCORES]
        perm[:, p] = ranks
        lps.append(int(live[ranks].max()))
    return perm, tuple(lps)


def _host_prep(joint_embedding, delta_embedding, Wq, Wk, Wv, traj_length):
    joint_embedding = np.asarray(joint_embedding, dtype=np.float32)
    delta_embedding = np.asarray(delta_embedding, dtype=np.float32)
    valid = (np.arange(L)[None, :] < np.asarray(traj_length)[:, None]
             ).astype(np.float32)
    perm, lps = _schedule(traj_length)

    dsum = delta_embedding.sum(axis=-1, dtype=np.float32).astype(np.float16)
    x16 = joint_embedding.astype(np.float16)
    ident = np.eye(128, dtype=np.float16)
    wq16 = np.asarray(Wq, dtype=np.float16)
    wk16 = np.asarray(Wk, dtype=np.float16)
    wv16 = np.asarray(Wv, dtype=np.float16)

    in_maps = []
    for c in range(NCORES):
        bs = perm[c]
        xT = np.ascontiguousarray(x16[bs].transpose(0, 2, 1))
        ds = np.ascontiguousarray(dsum[bs])
        valp = np.ascontiguousarray(
            valid[bs].reshape(BPC, NLT, 128).transpose(2, 0, 1))
        in_maps.append({
            "xT": xT,
            "ds": ds,
            "wq": wq16, "wk": wk16, "wv": wv16,
            "val": valp,
            "ident": ident,
        })
    return in_maps


def kernel(joint_embedding, delta_embedding, Wq, Wk, Wv, traj_length):
    from concourse.bass_utils import run_bass_kernel_spmd

    perm, lps = _schedule(traj_length)
    nc = _get_compiled(lps)
    in_maps = _host_prep(joint_embedding, delta_embedding, Wq, Wk, Wv,
                         traj_length)
    res = run_bass_kernel_spmd(nc, in_maps, core_ids=list(range(NCORES)))
    out = np.empty((B, L, O), dtype=np.float32)
    for c in range(NCORES):
        for p in range(BPC):
            out[perm[c][p]] = res.results[c]["out"][p].astype(np.float32)
    return out


# revision 6
# speedup vs baseline: 4248.3514x; 1.2020x over previous
"""CTR self-attention kernel for Trainium2 (8 NeuronCores, data-parallel
over batch; 4 batches per core, scheduled so per-position live-tile maxima
are minimal).

Reference computation (per batch b, L=1024, E=O=512, K=4):
    delta = delta_embedding.sum(-1)                       # [L, L]
    valid[i] = i < traj_length[b]
    mask = outer(valid, valid)
    q, k, v = X @ Wq, X @ Wk, X @ Wv                      # [L, O]
    scores = q @ k.T + delta                              # [L, L]
    attn = softmax(scores, axis=-1) * mask                # post-softmax mask
    out = attn @ v                                        # [L, O]

Design:
  - delta summed over K on the host -> [L, L] fp16 (4x less HBM traffic than
    the [L, K, L] bf16 layout); added into the scores PSUM group via an
    identity matmul (1024 PE rows/i-tile instead of 4096)
  - every matmul operand is fp16: same 11-bit mantissa as fp32r, so no
    accuracy loss vs the fp32r baseline, but half the SBUF/HBM bytes, FWL
    weight loads (fp32r disqualifies FWL), and no fp32 streaming penalty
  - mask structure exploited: with t = traj_length, live = ceil(t/128),
    q-projection / attention i-tiles / p-transposes / attn@v j-tiles all
    process only live tiles (k and the scores row stay full width: the
    post-softmax mask means the denominator includes all 1024 columns);
    column mask folded into v (zeroed rows), row mask into the 1/den scale
  - softmax: DVE negated rowmax -> ScalarE Exp(bias=-max) with fused row-sum
    accumulator -> reciprocal; p written as fp16, transposed 128x128 on the
    PE into one fp16 PSUM bank, one ScalarE copy out
  - PSUM: 3x[128,1024]f32 score accumulators (6 banks; also reused by the
    projection jobs) + a shared 2-slot pool for transpose/attn@v tiles
  - output stored fp16, widened to fp32 on the host
  - DMA: both HWDGE rings round-robin; loads ordered by first use; delta
    prefetched 8 i-tiles deep
"""

import os

import numpy as np

B, L, E, O, KD = 32, 1024, 1024 // 2, 512, 4
NCORES = 8
BPC = B // NCORES  # batches per core

NLT = L // 128   # 8 l-tiles
NET = E // 128   # 4 e-tiles
NOT = O // 128   # 4 o-tiles
NJC = L // 512   # 2 chunks of 512 along the free dim

_compiled = {}


def _chunks(total, maxc=512):
    out = []
    pos = 0
    while pos < total:
        c = min(maxc, total - pos)
        out.append((pos, c))
        pos += c
    return out


def _build(reps=1, lps=(8, 8, 8, 8)):
    from contextlib import ExitStack

    import concourse.bass as bass
    import concourse.tile as tile
    from concourse import bacc, mybir

    FP32 = mybir.dt.float32
    FP16 = mybir.dt.float16
    AX = mybir.AxisListType
    ALU = mybir.AluOpType
    ACTF = mybir.ActivationFunctionType

    nc = bacc.Bacc("TRN2", target_bir_lowering=False, debug=False,
                   num_devices=NCORES)

    xT_d = nc.dram_tensor("xT", (BPC, E, L), FP16, kind="ExternalInput")
    ds_d = nc.dram_tensor("ds", (BPC, L, L), FP16, kind="ExternalInput")
    wm_d = nc.dram_tensor("wm", (E, E), FP16, kind="ExternalInput")
    wv_d = nc.dram_tensor("wv", (E, O), FP16, kind="ExternalInput")
    val_d = nc.dram_tensor("val", (128, BPC, NLT), FP32, kind="ExternalInput")
    id_d = nc.dram_tensor("ident", (128, 128), FP16, kind="ExternalInput")
    out_d = nc.dram_tensor("out", (BPC, L, O), FP16, kind="ExternalOutput")

    with tile.TileContext(nc) as tc, ExitStack() as ctx:
        cpool = ctx.enter_context(tc.tile_pool(name="const", bufs=1))
        xpool = ctx.enter_context(tc.tile_pool(name="xt", bufs=2))
        qkpool = ctx.enter_context(tc.tile_pool(name="qk", bufs=2))
        vpool = ctx.enter_context(tc.tile_pool(name="v", bufs=2))
        dpool = ctx.enter_context(tc.tile_pool(name="delta", bufs=8))
        ppool = ctx.enter_context(tc.tile_pool(name="p", bufs=3))
        ptpool = ctx.enter_context(tc.tile_pool(name="pt", bufs=3))
        opool = ctx.enter_context(tc.tile_pool(name="osb", bufs=3))
        smpool = ctx.enter_context(tc.tile_pool(name="small", bufs=8))
        # PSUM: scores accumulators 2x[128,1024]f32 (4 banks), transposes
        # 2x[128,1024]f16 (2 banks), attn@v out 2x[128,512]f32 (2 banks)
        scps = ctx.enter_context(tc.tile_pool(name="scps", bufs=3, space="PSUM"))
        trps = ctx.enter_context(tc.tile_pool(name="trps", bufs=2, space="PSUM"))
        ops = trps

        rings = [nc.sync, nc.scalar]
        ring_i = [0]

        def dma(out_ap, in_ap):
            eng = rings[ring_i[0] % 2]
            ring_i[0] += 1
            eng.dma_start(out_ap, in_ap)

        # first batch's xT first: the first projection matmuls need
        # wq + xt[et0]; issuing xt ahead of the bulk weight loads trims the
        # startup bubble
        xt0 = xpool.tile([128, NET, L], FP16, tag="xt")
        wm_t = cpool.tile([128, NET, E], FP16, tag="wm")
        wv_t = cpool.tile([128, NET, O], FP16, tag="wv")
        id_t = cpool.tile([128, 128], FP16, tag="ident")
        vl = cpool.tile([128, BPC, NLT], FP32, tag="vl")
        # load order tracks first use: y = x@M projections (wm + xt0) start
        # the kernel, v/ident/val later
        dma(xt0[:, 0, :], xT_d[0, 0:128, :])
        dma(wm_t[:], wm_d[:].rearrange("(et p) o -> p et o", p=128))
        for et in range(1, NET):
            dma(xt0[:, et, :], xT_d[0, et * 128:(et + 1) * 128, :])
        dma(wv_t[:], wv_d[:].rearrange("(et p) o -> p et o", p=128))
        dma(id_t[:], id_d[:])
        dma(vl[:], val_d[:])

        for rep in range(reps):
            for b in range(BPC):
                live = lps[b]
                # ---- load xT for this batch ----
                if rep == 0 and b == 0:
                    xt = xt0
                else:
                    xt = xpool.tile([128, NET, L], FP16, tag="xt")
                    for et in range(NET):
                        dma(xt[:, et, :], xT_d[b, et * 128:(et + 1) * 128, :])

                # ---- projections (paired accumulators alternate PSUM
                # banks so the PE stays pipelined) ----
                yT = qkpool.tile([128, NET, L], FP16, tag="yT")
                v_t = vpool.tile([128, NLT, O], FP16, tag="v")
                qch = _chunks(live * 128)
                jobs = [("y", et2, c0, cn) for et2 in range(NET)
                        for (c0, cn) in qch]
                jobs += [("v", lt, 0, 512) for lt in range(live)]
                for j0 in range(0, len(jobs), 2):
                    pair = jobs[j0:j0 + 2]
                    accs = []
                    for pi in range(len(pair)):
                        acc = scps.tile([128, 1024], FP32, tag="sc",
                                        name=f"acc_{b}_{j0}_{pi}")
                        accs.append(acc)
                    for et in range(NET):
                        for pi, (kind, a1, c0, cn) in enumerate(pair):
                            if kind == "y":
                                nc.tensor.matmul(
                                    accs[pi][:, 0:cn],
                                    wm_t[:, et, a1 * 128:(a1 + 1) * 128],
                                    xt[:, et, c0:c0 + cn],
                                    start=(et == 0), stop=(et == NET - 1),
                                )
                            else:
                                nc.tensor.matmul(
                                    accs[pi][:, 0:512],
                                    xt[:, et, a1 * 128:(a1 + 1) * 128],
                                    wv_t[:, et, :],
                                    start=(et == 0), stop=(et == NET - 1),
                                )
                    for pi, (kind, a1, c0, cn) in enumerate(pair):
                        if kind == "y":
                            nc.vector.tensor_copy(
                                yT[:, a1, c0:c0 + cn], accs[pi][:, 0:cn])
                        else:
                            # fold column mask: zero rows j >= t in the
                            # (only partial) last live tile
                            nc.scalar.activation(
                                v_t[:, a1, :], accs[pi][:, 0:512], ACTF.Copy,
                                bias=0.0, scale=vl[:, b, a1:a1 + 1],
                            )

                # ---- attention, one i-tile (128 queries) at a time ----
                for it in range(live):
                    dl = dpool.tile([128, 1024], FP16, tag="dl",
                                    name=f"dl_{b}_{it}")
                    dma(dl[:], ds_d[b, it * 128:(it + 1) * 128, :])

                    sc = scps.tile([128, 1024], FP32, tag="sc",
                                   name=f"sc_{b}_{it}")
                    # scores = y @ x.T  (y = x @ (Wq Wk^T), so this is
                    # q @ k.T with the k-projection folded away)
                    for et2 in range(NET):
                        for jc in range(NJC):
                            nc.tensor.matmul(
                                sc[:, jc * 512:(jc + 1) * 512],
                                yT[:, et2, it * 128:(it + 1) * 128],
                                xt[:, et2, jc * 512:(jc + 1) * 512],
                                start=(et2 == 0), stop=False,
                                skip_group_check=True,
                            )
                    # += delta via identity matmul
                    for jc in range(NJC):
                        nc.tensor.matmul(
                            sc[:, jc * 512:(jc + 1) * 512],
                            id_t[:],
                            dl[:, jc * 512:(jc + 1) * 512],
                            start=False, stop=True,
                            skip_group_check=True,
                        )
                    # softmax over the full row; mask folded in afterwards
                    nm = smpool.tile([128, 1], FP32, tag="nm",
                                     name=f"nm_{b}_{it}")
                    nc.vector.tensor_reduce(nm[:], sc[:], axis=AX.X,
                                            op=ALU.max, negate=True)
                    p = ppool.tile([128, 1024], FP16, tag="p",
                                   name=f"p_{b}_{it}")
                    den = smpool.tile([128, 1], FP32, tag="den",
                                      name=f"den_{b}_{it}")
                    nc.scalar.activation(p[:], sc[:], ACTF.Exp, bias=nm[:],
                                         scale=1.0, accum_out=den[:])
                    rs = smpool.tile([128, 1], FP32, tag="rs",
                                     name=f"rs_{b}_{it}")
                    nc.vector.reciprocal(rs[:], den[:])

                    # transpose live j-tiles of p into one fp16 PSUM bank
                    ptp = trps.tile([128, 1024], FP16, tag="ptp",
                                    name=f"ptp_{b}_{it}")
                    for jt in range(live):
                        nc.tensor.transpose(
                            ptp[:, jt * 128:(jt + 1) * 128],
                            p[:, jt * 128:(jt + 1) * 128],
                            id_t[:],
                        )
                    pT = ptpool.tile([128, 1024], FP16, tag="pT",
                                     name=f"pT_{b}_{it}")
                    h1 = ((live + 1) // 2) * 128
                    nc.scalar.copy(pT[:, 0:h1], ptp[:, 0:h1])
                    if live * 128 > h1:
                        nc.scalar.copy(pT[:, h1:live * 128],
                                       ptp[:, h1:live * 128])

                    # out = (p/den * rowmask) @ v -- only live j-tiles
                    op = ops.tile([128, 512], FP32, tag="ptp",
                                  name=f"op_{b}_{it}")
                    for jt in range(live):
                        nc.tensor.matmul(
                            op[:],
                            pT[:, jt * 128:(jt + 1) * 128],
                            v_t[:, jt, :],
                            start=(jt == 0), stop=(jt == live - 1),
                        )
                    ob = opool.tile([128, 512], FP16, tag="ob",
                                    name=f"ob_{b}_{it}")
                    nc.vector.tensor_scalar(ob[:], op[:], rs[:],
                                            vl[:, b, it:it + 1],
                                            op0=ALU.mult, op1=ALU.mult)
                    nc.sync.dma_start(
                        out_d[b, it * 128:(it + 1) * 128, :], ob[:]
                    )

    nc.compile()
    return nc


def _get_compiled(lps=(8, 8, 8, 8)):
    lps = tuple(lps)
    if lps not in _compiled:
        _compiled[lps] = _build(
            reps=int(os.environ.get("CTR_KERNEL_REPS", "1")), lps=lps)
    return _compiled[lps]


def _schedule(traj_length):
    """Assign batches to (core, position) so that position-wise max live
    i-tile counts are minimal; returns (perm, lps) with perm[c][p] = batch."""
    traj = np.asarray(traj_length)
    live = np.minimum((traj.astype(np.int64) + 127) // 128, L // 128)
    order = np.argsort(-live, kind="stable")
    perm = np.empty((NCORES, BPC), dtype=np.int64)
    lps = []
    for p in range(BPC):
        ranks = order[p * NCORES:(p + 1) * NCORES]
        perm[:, p] = ranks
        lps.append(int(live[ranks].max()))
    return perm, tuple(lps)


def _host_prep(joint_embedding, delta_embedding, Wq, Wk, Wv, traj_length):
    joint_embedding = np.asarray(joint_embedding, dtype=np.float32)
    delta_embedding = np.asarray(delta_embedding, dtype=np.float32)
    valid = (np.arange(L)[None, :] < np.asarray(traj_length)[:, None]
             ).astype(np.float32)
    perm, lps = _schedule(traj_length)

    dsum = delta_embedding.sum(axis=-1, dtype=np.float32).astype(np.float16)
    x16 = joint_embedding.astype(np.float16)
    ident = np.eye(128, dtype=np.float16)
    wm16 = (np.asarray(Wq, dtype=np.float32)
            @ np.asarray(Wk, dtype=np.float32).T).astype(np.float16)
    wv16 = np.asarray(Wv, dtype=np.float16)

    in_maps = []
    for c in range(NCORES):
        bs = perm[c]
        xT = np.ascontiguousarray(x16[bs].transpose(0, 2, 1))
        ds = np.ascontiguousarray(dsum[bs])
        valp = np.ascontiguousarray(
            valid[bs].reshape(BPC, NLT, 128).transpose(2, 0, 1))
        in_maps.append({
            "xT": xT,
            "ds": ds,
            "wm": wm16, "wv": wv16,
            "val": valp,
            "ident": ident,
        })
    return in_maps


def kernel(joint_embedding, delta_embedding, Wq, Wk, Wv, traj_length):
    from concourse.bass_utils import run_bass_kernel_spmd

    perm, lps = _schedule(traj_length)
    nc = _get_compiled(lps)
    in_maps = _host_prep(joint_embedding, delta_embedding, Wq, Wk, Wv,
                         traj_length)
    res = run_bass_kernel_spmd(nc, in_maps, core_ids=list(range(NCORES)))
    out = np.empty((B, L, O), dtype=np.float32)
    for c in range(NCORES):
        for p in range(BPC):
            out[perm[c][p]] = res.results[c]["out"][p].astype(np.float32)
    return out


# revision 10
# speedup vs baseline: 4311.8532x; 1.0149x over previous
"""CTR self-attention kernel for Trainium2 (8 NeuronCores, data-parallel
over batch; 4 batches per core, scheduled so per-position live-tile maxima
are minimal).

Reference computation (per batch b, L=1024, E=O=512, K=4):
    delta = delta_embedding.sum(-1)                       # [L, L]
    valid[i] = i < traj_length[b];  mask = outer(valid, valid)
    q, k, v = X @ Wq, X @ Wk, X @ Wv
    out = (softmax(q @ k.T + delta, axis=-1) * mask) @ v  # post-softmax mask

Design:
  - k-projection eliminated algebraically: scores = q@k.T = x@(Wq Wk^T)@x.T,
    so the host precomputes M = Wq@Wk^T and the device computes y = x@M
    (same cost as the old q-projection) and contracts scores directly
    against the resident xT tile -- saves 16384 PE rows per batch
  - delta summed over K on the host -> [L, L] fp16 (4x less HBM traffic than
    [L, K, L] bf16); added into the scores PSUM group via an identity matmul
  - every matmul operand is fp16: same 11-bit mantissa as fp32r, half the
    bytes, FWL-eligible weight loads (fp32r is not), no fp32 stream penalty
  - mask structure exploited: with live = ceil(t/128), the y-projection,
    attention i-loop, p-transposes, and attn@v all process only live tiles
    (scores/denominator stay full-width: the post-softmax mask means invalid
    columns still feed the softmax denominator); column mask folded into v
    (zeroed rows), row mask fused into the output scale (one tensor_scalar)
  - softmax: DVE negated rowmax -> ScalarE Exp(bias=-max) with fused row-sum
    accumulator -> reciprocal; p fp16, transposed 128x128 on the PE into one
    fp16 PSUM bank, one ScalarE copy out
  - PSUM: 3x[128,1024]f32 score accumulators (6 banks, reused by projection
    jobs) + a shared 2-slot pool for transpose/attn@v tiles
  - output stored fp16, widened to fp32 on the host
  - DMA: both HWDGE rings round-robin, loads ordered by first use, delta
    prefetched 8 i-tiles deep
"""

import os

import numpy as np

B, L, E, O, KD = 32, 1024, 1024 // 2, 512, 4
NCORES = 8
BPC = B // NCORES  # batches per core

NLT = L // 128   # 8 l-tiles
NET = E // 128   # 4 e-tiles
NOT = O // 128   # 4 o-tiles
NJC = L // 512   # 2 chunks of 512 along the free dim

_compiled = {}


def _chunks(total, maxc=512):
    out = []
    pos = 0
    while pos < total:
        c = min(maxc, total - pos)
        out.append((pos, c))
        pos += c
    return out


def _build(reps=1, lps=(8, 8, 8, 8)):
    from contextlib import ExitStack

    import concourse.bass as bass
    import concourse.tile as tile
    from concourse import bacc, mybir

    FP32 = mybir.dt.float32
    FP16 = mybir.dt.float16
    AX = mybir.AxisListType
    ALU = mybir.AluOpType
    ACTF = mybir.ActivationFunctionType

    nc = bacc.Bacc("TRN2", target_bir_lowering=False, debug=False,
                   num_devices=NCORES)

    xT_d = nc.dram_tensor("xT", (BPC, E, L), FP16, kind="ExternalInput")
    ds_d = nc.dram_tensor("ds", (BPC, L, L), FP16, kind="ExternalInput")
    wm_d = nc.dram_tensor("wm", (E, E), FP16, kind="ExternalInput")
    wv_d = nc.dram_tensor("wv", (E, O), FP16, kind="ExternalInput")
    val_d = nc.dram_tensor("val", (128, BPC, NLT), FP32, kind="ExternalInput")
    id_d = nc.dram_tensor("ident", (128, 128), FP16, kind="ExternalInput")
    out_d = nc.dram_tensor("out", (BPC, L, O), FP16, kind="ExternalOutput")

    with tile.TileContext(nc) as tc, ExitStack() as ctx:
        cpool = ctx.enter_context(tc.tile_pool(name="const", bufs=1))
        xpool = ctx.enter_context(tc.tile_pool(name="xt", bufs=2))
        qkpool = ctx.enter_context(tc.tile_pool(name="qk", bufs=2))
        vpool = ctx.enter_context(tc.tile_pool(name="v", bufs=2))
        dpool = ctx.enter_context(tc.tile_pool(name="delta", bufs=8))
        ppool = ctx.enter_context(tc.tile_pool(name="p", bufs=3))
        ptpool = ctx.enter_context(tc.tile_pool(name="pt", bufs=3))
        opool = ctx.enter_context(tc.tile_pool(name="osb", bufs=3))
        smpool = ctx.enter_context(tc.tile_pool(name="small", bufs=8))
        # PSUM: scores accumulators 2x[128,1024]f32 (4 banks), transposes
        # 2x[128,1024]f16 (2 banks), attn@v out 2x[128,512]f32 (2 banks)
        scps = ctx.enter_context(tc.tile_pool(name="scps", bufs=3, space="PSUM"))
        trps = ctx.enter_context(tc.tile_pool(name="trps", bufs=2, space="PSUM"))
        ops = trps

        rings = [nc.sync, nc.scalar]
        ring_i = [0]

        def dma(out_ap, in_ap):
            eng = rings[ring_i[0] % 2]
            ring_i[0] += 1
            eng.dma_start(out_ap, in_ap)

        # first batch's xT first: the first projection matmuls need
        # wq + xt[et0]; issuing xt ahead of the bulk weight loads trims the
        # startup bubble
        xt0 = xpool.tile([128, NET, L], FP16, tag="xt")
        wm_t = cpool.tile([128, NET, E], FP16, tag="wm")
        wv_t = cpool.tile([128, NET, O], FP16, tag="wv")
        id_t = cpool.tile([128, 128], FP16, tag="ident")
        vl = cpool.tile([128, BPC, NLT], FP32, tag="vl")
        # load order tracks first use: y = x@M projections (wm + xt0)
        # start the kernel, so interleave per-et wm/xt0 slices (the first
        # accumulation step needs only the et=0 pair), then v/ident/val
        for et in range(NET):
            dma(wm_t[:, et, :], wm_d[et * 128:(et + 1) * 128, :])
            dma(xt0[:, et, :], xT_d[0, et * 128:(et + 1) * 128, :])
        dma(wv_t[:], wv_d[:].rearrange("(et p) o -> p et o", p=128))
        dma(id_t[:], id_d[:])
        dma(vl[:], val_d[:])

        for rep in range(reps):
            for b in range(BPC):
                live = lps[b]
                # ---- load xT for this batch ----
                if rep == 0 and b == 0:
                    xt = xt0
                else:
                    xt = xpool.tile([128, NET, L], FP16, tag="xt")
                    for et in range(NET):
                        dma(xt[:, et, :], xT_d[b, et * 128:(et + 1) * 128, :])

                # ---- projections (paired accumulators alternate PSUM
                # banks so the PE stays pipelined) ----
                yT = qkpool.tile([128, NET, L], FP16, tag="yT")
                v_t = vpool.tile([128, NLT, O], FP16, tag="v")
                # prime the delta prefetch pipeline during the projections
                dls = {}
                for it in range(min(2, live)):
                    dl = dpool.tile([128, 1024], FP16, tag="dl",
                                    name=f"dl_{b}_{it}")
                    dma(dl[:], ds_d[b, it * 128:(it + 1) * 128, :])
                    dls[it] = dl
                qch = _chunks(live * 128)
                jobs = [("y", et2, c0, cn) for et2 in range(NET)
                        for (c0, cn) in qch]
                jobs += [("v", lt, 0, 512) for lt in range(live)]
                for j0 in range(0, len(jobs), 2):
                    pair = jobs[j0:j0 + 2]
                    accs = []
                    for pi in range(len(pair)):
                        acc = scps.tile([128, 1024], FP32, tag="sc",
                                        name=f"acc_{b}_{j0}_{pi}")
                        accs.append(acc)
                    for et in range(NET):
                        for pi, (kind, a1, c0, cn) in enumerate(pair):
                            if kind == "y":
                                nc.tensor.matmul(
                                    accs[pi][:, 0:cn],
                                    wm_t[:, et, a1 * 128:(a1 + 1) * 128],
                                    xt[:, et, c0:c0 + cn],
                                    start=(et == 0), stop=(et == NET - 1),
                                )
                            else:
                                nc.tensor.matmul(
                                    accs[pi][:, 0:512],
                                    xt[:, et, a1 * 128:(a1 + 1) * 128],
                                    wv_t[:, et, :],
                                    start=(et == 0), stop=(et == NET - 1),
                                )
                    for pi, (kind, a1, c0, cn) in enumerate(pair):
                        if kind == "y":
                            nc.vector.tensor_copy(
                                yT[:, a1, c0:c0 + cn], accs[pi][:, 0:cn])
                        else:
                            # fold column mask: zero rows j >= t in the
                            # (only partial) last live tile
                            nc.scalar.activation(
                                v_t[:, a1, :], accs[pi][:, 0:512], ACTF.Copy,
                                bias=0.0, scale=vl[:, b, a1:a1 + 1],
                            )

                # ---- attention, one i-tile (128 queries) at a time ----
                for it in range(live):
                    if it in dls:
                        dl = dls[it]
                    else:
                        dl = dpool.tile([128, 1024], FP16, tag="dl",
                                        name=f"dl_{b}_{it}")
                        dma(dl[:], ds_d[b, it * 128:(it + 1) * 128, :])

                    sc = scps.tile([128, 1024], FP32, tag="sc",
                                   name=f"sc_{b}_{it}")
                    # scores = y @ x.T  (y = x @ (Wq Wk^T), so this is
                    # q @ k.T with the k-projection folded away)
                    for et2 in range(NET):
                        for jc in range(NJC):
                            nc.tensor.matmul(
                                sc[:, jc * 512:(jc + 1) * 512],
                                yT[:, et2, it * 128:(it + 1) * 128],
                                xt[:, et2, jc * 512:(jc + 1) * 512],
                                start=(et2 == 0), stop=False,
                                skip_group_check=True,
                            )
                    # += delta via identity matmul
                    for jc in range(NJC):
                        nc.tensor.matmul(
                            sc[:, jc * 512:(jc + 1) * 512],
                            id_t[:],
                            dl[:, jc * 512:(jc + 1) * 512],
                            start=False, stop=True,
                            skip_group_check=True,
                        )
                    # softmax over the full row; mask folded in afterwards
                    nm = smpool.tile([128, 1], FP32, tag="nm",
                                     name=f"nm_{b}_{it}")
                    nc.vector.tensor_reduce(nm[:], sc[:], axis=AX.X,
                                            op=ALU.max, negate=True)
                    p = ppool.tile([128, 1024], FP16, tag="p",
                                   name=f"p_{b}_{it}")
                    den = smpool.tile([128, 1], FP32, tag="den",
                                      name=f"den_{b}_{it}")
                    nc.scalar.activation(p[:], sc[:], ACTF.Exp, bias=nm[:],
                                         scale=1.0, accum_out=den[:])
                    rs = smpool.tile([128, 1], FP32, tag="rs",
                                     name=f"rs_{b}_{it}")
                    nc.vector.reciprocal(rs[:], den[:])

                    # transpose live j-tiles of p into one fp16 PSUM bank
                    ptp = trps.tile([128, 1024], FP16, tag="ptp",
                                    name=f"ptp_{b}_{it}")
                    for jt in range(live):
                        nc.tensor.transpose(
                            ptp[:, jt * 128:(jt + 1) * 128],
                            p[:, jt * 128:(jt + 1) * 128],
                            id_t[:],
                        )
                    pT = ptpool.tile([128, 1024], FP16, tag="pT",
                                     name=f"pT_{b}_{it}")
                    nc.scalar.copy(pT[:, 0:live * 128], ptp[:, 0:live * 128])

                    # out = (p/den * rowmask) @ v -- only live j-tiles
                    op = ops.tile([128, 512], FP32, tag="ptp",
                                  name=f"op_{b}_{it}")
                    for jt in range(live):
                        nc.tensor.matmul(
                            op[:],
                            pT[:, jt * 128:(jt + 1) * 128],
                            v_t[:, jt, :],
                            start=(jt == 0), stop=(jt == live - 1),
                        )
                    ob = opool.tile([128, 512], FP16, tag="ob",
                                    name=f"ob_{b}_{it}")
                    nc.vector.tensor_scalar(ob[:], op[:], rs[:],
                                            vl[:, b, it:it + 1],
                                            op0=ALU.mult, op1=ALU.mult)
                    nc.sync.dma_start(
                        out_d[b, it * 128:(it + 1) * 128, :], ob[:]
                    )

    nc.compile()
    return nc


def _get_compiled(lps=(8, 8, 8, 8)):
    lps = tuple(lps)
    if lps not in _compiled:
        _compiled[lps] = _build(
            reps=int(os.environ.get("CTR_KERNEL_REPS", "1")), lps=lps)
    return _compiled[lps]


def _schedule(traj_length):
    """Assign batches to (core, position) so that position-wise max live
    i-tile counts are minimal; returns (perm, lps) with perm[c][p] = batch."""
    traj = np.asarray(traj_length)
    live = np.minimum((traj.astype(np.int64) + 127) // 128, L // 128)
    order = np.argsort(-live, kind="stable")
    perm = np.empty((NCORES, BPC), dtype=np.int64)
    lps = []
    for p in range(BPC):
        ranks = order[p * NCORES:(p + 1) * NCORES]
        perm[:, p] = ranks
        lps.append(int(live[ranks].max()))
    return perm, tuple(lps)


def _host_prep(joint_embedding, delta_embedding, Wq, Wk, Wv, traj_length):
    joint_embedding = np.asarray(joint_embedding, dtype=np.float32)
    delta_embedding = np.asarray(delta_embedding, dtype=np.float32)
    valid = (np.arange(L)[None, :] < np.asarray(traj_length)[:, None]
             ).astype(np.float32)
    perm, lps = _schedule(traj_length)

    dsum = delta_embedding.sum(axis=-1, dtype=np.float32).astype(np.float16)
    x16 = joint_embedding.astype(np.float16)
    ident = np.eye(128, dtype=np.float16)
    wm16 = (np.asarray(Wq, dtype=np.float32)
            @ np.asarray(Wk, dtype=np.float32).T).astype(np.float16)
    wv16 = np.asarray(Wv, dtype=np.float16)

    in_maps = []
    for c in range(NCORES):
        bs = perm[c]
        xT = np.ascontiguousarray(x16[bs].transpose(0, 2, 1))
        ds = np.ascontiguousarray(dsum[bs])
        valp = np.ascontiguousarray(
            valid[bs].reshape(BPC, NLT, 128).transpose(2, 0, 1))
        in_maps.append({
            "xT": xT,
            "ds": ds,
            "wm": wm16, "wv": wv16,
            "val": valp,
            "ident": ident,
        })
    return in_maps


def kernel(joint_embedding, delta_embedding, Wq, Wk, Wv, traj_length):
    from concourse.bass_utils import run_bass_kernel_spmd

    perm, lps = _schedule(traj_length)
    nc = _get_compiled(lps)
    in_maps = _host_prep(joint_embedding, delta_embedding, Wq, Wk, Wv,
                         traj_length)
    res = run_bass_kernel_spmd(nc, in_maps, core_ids=list(range(NCORES)))
    out = np.empty((B, L, O), dtype=np.float32)
    for c in range(NCORES):
        for p in range(BPC):
            out[perm[c][p]] = res.results[c]["out"][p].astype(np.float32)
    return out
